# revision 17
# baseline (speedup 1.0000x reference)
"""AssocScan Trainium2 kernel: out[:, t] = gates[:, t] * out[:, t-1] + inputs[:, t].

Strategy: the recurrence is independent per (b, d) lane (B*D = 4096 lanes,
N = 4096 steps). The DVE `tensor_tensor_scan` instruction computes exactly
this recurrence along the free dimension, one lane per partition.

Sharding: lanes are split evenly across the 8 NeuronCores (512 lanes each).
During host-side sharding the (B, N, D) inputs are transposed to lane-major
(B*D, N) so every device DMA is fully contiguous (time series per lane
contiguous in DRAM); each core loads its shard with two 8 MiB DMAs, scans
4 tiles of [128 lanes, 4096 steps] in place on the VectorEngine, and
streams the results back.
"""

import sys

import numpy as np

for _p in ("/opt/trn_rl_repo", "/opt/pypackages"):
    if _p not in sys.path:
        sys.path.append(_p)

import concourse.bacc as bacc
import concourse.mybir as mybir
from concourse.bass_utils import run_bass_kernel_spmd
from concourse.tile import TileContext

B, N, D = 4, 4096, 1024
N_CORES = 8
LANES = B * D                        # 4096 independent (b, d) lanes
LANES_PER_CORE = LANES // N_CORES    # 512
P = 128                              # SBUF partitions
TILES_PER_CORE = LANES_PER_CORE // P # 4

TRACE = False       # test harness sets True to capture a neuron-profile trace
USE_BF16 = False    # bf16 input storage (halves load bytes); fp32 scan state
_result_info = {}   # exec_time_ns / trace path from the last run

# Per chunk: (n load splits, n scan/store splits) along N. Finer splits on
# the first chunk start the DVE scan chain sooner (its loads are smaller);
# finer splits on the last chunk shorten the exposed tail. Middle chunks
# keep 1 MiB loads for full DMA efficiency — DVE is the busy resource there.
_PLAN = [(4, 4), (1, 2), (1, 2), (2, 2)]


def _build() -> bacc.Bacc:
    in_dt = mybir.dt.bfloat16 if USE_BF16 else mybir.dt.float32
    nc = bacc.Bacc()
    g = nc.dram_tensor("gates", [LANES_PER_CORE, N], in_dt, kind="ExternalInput")
    x = nc.dram_tensor("inputs", [LANES_PER_CORE, N], in_dt, kind="ExternalInput")
    o = nc.dram_tensor(
        "out", [LANES_PER_CORE, N], mybir.dt.float32, kind="ExternalOutput"
    )
    with TileContext(nc) as tc:
        with tc.tile_pool(name="pool", bufs=3) as pool:
            for i, (nload, nscan) in enumerate(_PLAN):
                rows = slice(i * P, (i + 1) * P)
                gt = pool.tile([P, N], in_dt, tag="g")
                xt = pool.tile([P, N], in_dt, tag="x")
                # The scan's internal state is fp32 regardless of operand
                # dtype; with a separate fp32 output tile the only bf16 loss
                # is input quantization. fp32 path scans in place (the DVE
                # write trails the read by the pipeline depth).
                if USE_BF16:
                    ot = pool.tile([P, N], mybir.dt.float32, tag="o", name="ot")
                else:
                    ot = xt
                # Interleaved loads on the sync-engine HWDGE ring: FIFO
                # drain makes completions arrive in exactly scan order, so
                # each scan segment waits only for its own bytes.
                L = N // nload
                for s in range(nload):
                    cols = slice(s * L, (s + 1) * L)
                    nc.sync.dma_start(out=gt[:, cols], in_=g[rows, cols])
                    nc.sync.dma_start(out=xt[:, cols], in_=x[rows, cols])
                S = N // nscan
                for s in range(nscan):
                    cols = slice(s * S, (s + 1) * S)
                    init = 0.0 if s == 0 else ot[:, s * S - 1 : s * S]
                    nc.vector.tensor_tensor_scan(
                        ot[:, cols],
                        gt[:, cols],
                        xt[:, cols],
                        init,
                        mybir.AluOpType.mult,
                        mybir.AluOpType.add,
                    )
                    # Stores ride the scalar-engine HWDGE ring so their
                    # waits never stall load issue on the sync ring.
                    nc.scalar.dma_start(out=o[rows, cols], in_=ot[:, cols])
    nc.compile()
    return nc


def kernel(gates: np.ndarray, inputs: np.ndarray) -> np.ndarray:
    gates = np.asarray(gates, dtype=np.float32)
    inputs = np.asarray(inputs, dtype=np.float32)

    # Host-side shard: (B, N, D) -> lane-major (B*D, N); row b*D + d is the
    # contiguous time series of lane (b, d).
    gt = np.ascontiguousarray(gates.transpose(0, 2, 1)).reshape(LANES, N)
    xt = np.ascontiguousarray(inputs.transpose(0, 2, 1)).reshape(LANES, N)
    if USE_BF16:
        import ml_dtypes

        gt = gt.astype(ml_dtypes.bfloat16)
        xt = xt.astype(ml_dtypes.bfloat16)

    in_maps = []
    for c in range(N_CORES):
        rows = slice(c * LANES_PER_CORE, (c + 1) * LANES_PER_CORE)
        in_maps.append({"gates": gt[rows], "inputs": xt[rows]})

    nc = _build()
    res = run_bass_kernel_spmd(
        nc, in_maps, core_ids=list(range(N_CORES)), trace=TRACE
    )
    _result_info["exec_time_ns"] = res.exec_time_ns
    _result_info["mean_exec_time_ns"] = res.mean_exec_time_ns
    _result_info["profile_json"] = res.profile_json
    _result_info["trace"] = (
        res.instructions_and_trace[1] if res.instructions_and_trace else None
    )

    out_t = np.concatenate([r["out"] for r in res.results], axis=0)  # (LANES, N)
    return np.ascontiguousarray(out_t.reshape(B, D, N).transpose(0, 2, 1))


# revision 18
# speedup vs baseline: 1.0203x; 1.0203x over previous
"""AssocScan Trainium2 kernel: out[:, t] = gates[:, t] * out[:, t-1] + inputs[:, t].

Strategy: the recurrence is independent per (b, d) lane (B*D = 4096 lanes,
N = 4096 steps). The DVE `tensor_tensor_scan` instruction computes exactly
this recurrence along the free dimension, one lane per partition.

Sharding: lanes are split evenly across the 8 NeuronCores (512 lanes each).
During host-side sharding the (B, N, D) inputs are transposed to lane-major
(B*D, N) so every device DMA is fully contiguous (time series per lane
contiguous in DRAM); each core loads its shard with two 8 MiB DMAs, scans
4 tiles of [128 lanes, 4096 steps] in place on the VectorEngine, and
streams the results back.
"""

import sys

import numpy as np

for _p in ("/opt/trn_rl_repo", "/opt/pypackages"):
    if _p not in sys.path:
        sys.path.append(_p)

import concourse.bacc as bacc
import concourse.mybir as mybir
from concourse.bass_utils import run_bass_kernel_spmd
from concourse.tile import TileContext

B, N, D = 4, 4096, 1024
N_CORES = 8
LANES = B * D                        # 4096 independent (b, d) lanes
LANES_PER_CORE = LANES // N_CORES    # 512
P = 128                              # SBUF partitions
TILES_PER_CORE = LANES_PER_CORE // P # 4

TRACE = False       # test harness sets True to capture a neuron-profile trace
USE_BF16 = False    # bf16 input storage (halves load bytes); fp32 scan state
_result_info = {}   # exec_time_ns / trace path from the last run

# Per chunk: (n load splits, n scan/store splits) along N. Finer splits on
# the first chunk start the DVE scan chain sooner (its loads are smaller);
# finer splits on the last chunk shorten the exposed tail. Middle chunks
# keep 1 MiB loads for full DMA efficiency — DVE is the busy resource there.
_PLAN = [(2, 2), (1, 2), (1, 2), (1, 2)]


def _build() -> bacc.Bacc:
    in_dt = mybir.dt.bfloat16 if USE_BF16 else mybir.dt.float32
    nc = bacc.Bacc()
    g = nc.dram_tensor("gates", [LANES_PER_CORE, N], in_dt, kind="ExternalInput")
    x = nc.dram_tensor("inputs", [LANES_PER_CORE, N], in_dt, kind="ExternalInput")
    o = nc.dram_tensor(
        "out", [LANES_PER_CORE, N], mybir.dt.float32, kind="ExternalOutput"
    )
    with TileContext(nc) as tc:
        with tc.tile_pool(name="pool", bufs=3) as pool:
            for i, (nload, nscan) in enumerate(_PLAN):
                rows = slice(i * P, (i + 1) * P)
                gt = pool.tile([P, N], in_dt, tag="g")
                xt = pool.tile([P, N], in_dt, tag="x")
                # The scan's internal state is fp32 regardless of operand
                # dtype; with a separate fp32 output tile the only bf16 loss
                # is input quantization. fp32 path scans in place (the DVE
                # write trails the read by the pipeline depth).
                if USE_BF16:
                    ot = pool.tile([P, N], mybir.dt.float32, tag="o", name="ot")
                else:
                    ot = xt
                # Interleaved loads on the sync-engine HWDGE ring: FIFO
                # drain makes completions arrive in exactly scan order, so
                # each scan segment waits only for its own bytes.
                L = N // nload
                for s in range(nload):
                    cols = slice(s * L, (s + 1) * L)
                    nc.sync.dma_start(out=gt[:, cols], in_=g[rows, cols])
                    nc.sync.dma_start(out=xt[:, cols], in_=x[rows, cols])
                S = N // nscan
                for s in range(nscan):
                    cols = slice(s * S, (s + 1) * S)
                    init = 0.0 if s == 0 else ot[:, s * S - 1 : s * S]
                    nc.vector.tensor_tensor_scan(
                        ot[:, cols],
                        gt[:, cols],
                        xt[:, cols],
                        init,
                        mybir.AluOpType.mult,
                        mybir.AluOpType.add,
                    )
                    # Stores ride the scalar-engine HWDGE ring so their
                    # waits never stall load issue on the sync ring.
                    nc.scalar.dma_start(out=o[rows, cols], in_=ot[:, cols])
    nc.compile()
    return nc


def kernel(gates: np.ndarray, inputs: np.ndarray) -> np.ndarray:
    gates = np.asarray(gates, dtype=np.float32)
    inputs = np.asarray(inputs, dtype=np.float32)

    # Host-side shard: (B, N, D) -> lane-major (B*D, N); row b*D + d is the
    # contiguous time series of lane (b, d).
    gt = np.ascontiguousarray(gates.transpose(0, 2, 1)).reshape(LANES, N)
    xt = np.ascontiguousarray(inputs.transpose(0, 2, 1)).reshape(LANES, N)
    if USE_BF16:
        import ml_dtypes

        gt = gt.astype(ml_dtypes.bfloat16)
        xt = xt.astype(ml_dtypes.bfloat16)

    in_maps = []
    for c in range(N_CORES):
        rows = slice(c * LANES_PER_CORE, (c + 1) * LANES_PER_CORE)
        in_maps.append({"gates": gt[rows], "inputs": xt[rows]})

    nc = _build()
    res = run_bass_kernel_spmd(
        nc, in_maps, core_ids=list(range(N_CORES)), trace=TRACE
    )
    _result_info["exec_time_ns"] = res.exec_time_ns
    _result_info["mean_exec_time_ns"] = res.mean_exec_time_ns
    _result_info["profile_json"] = res.profile_json
    _result_info["trace"] = (
        res.instructions_and_trace[1] if res.instructions_and_trace else None
    )

    out_t = np.concatenate([r["out"] for r in res.results], axis=0)  # (LANES, N)
    return np.ascontiguousarray(out_t.reshape(B, D, N).transpose(0, 2, 1))


# revision 20
# speedup vs baseline: 1.0726x; 1.0513x over previous
"""AssocScan Trainium2 kernel: out[:, t] = gates[:, t] * out[:, t-1] + inputs[:, t].

Strategy: the recurrence is independent per (b, d) lane (B*D = 4096 lanes,
N = 4096 steps). The DVE `tensor_tensor_scan` instruction computes exactly
this recurrence along the free dimension, one lane per partition.

Sharding: lanes are split evenly across the 8 NeuronCores (512 lanes each).
During host-side sharding the (B, N, D) inputs are transposed to lane-major
(B*D, N) so every device DMA is fully contiguous (time series per lane
contiguous in DRAM); each core loads its shard with two 8 MiB DMAs, scans
4 tiles of [128 lanes, 4096 steps] in place on the VectorEngine, and
streams the results back.
"""

import sys

import numpy as np

for _p in ("/opt/trn_rl_repo", "/opt/pypackages"):
    if _p not in sys.path:
        sys.path.append(_p)

import concourse.bacc as bacc
import concourse.mybir as mybir
from concourse.bass_utils import run_bass_kernel_spmd
from concourse.tile import TileContext

B, N, D = 4, 4096, 1024
N_CORES = 8
LANES = B * D                        # 4096 independent (b, d) lanes
LANES_PER_CORE = LANES // N_CORES    # 512
P = 128                              # SBUF partitions
TILES_PER_CORE = LANES_PER_CORE // P # 4

TRACE = False       # test harness sets True to capture a neuron-profile trace
USE_BF16 = False    # bf16 input storage (halves load bytes); fp32 scan state
_result_info = {}   # exec_time_ns / trace path from the last run

# Scan/store segment sizes along N per chunk. Loads are always whole-chunk
# (small DMAs drop well below line rate). The last chunk's final segments
# are smaller to shorten the exposed store tail after the last scan.
_PLAN = [
    [2048, 2048],
    [2048, 2048],
    [2048, 2048],
    [2048, 1024, 1024],
]


def _build() -> bacc.Bacc:
    in_dt = mybir.dt.bfloat16 if USE_BF16 else mybir.dt.float32
    nc = bacc.Bacc()
    g = nc.dram_tensor("gates", [LANES_PER_CORE, N], in_dt, kind="ExternalInput")
    x = nc.dram_tensor("inputs", [LANES_PER_CORE, N], in_dt, kind="ExternalInput")
    o = nc.dram_tensor(
        "out", [LANES_PER_CORE, N], mybir.dt.float32, kind="ExternalOutput"
    )
    with TileContext(nc) as tc:
        with tc.tile_pool(name="pool", bufs=3) as pool:
            for i, segs in enumerate(_PLAN):
                rows = slice(i * P, (i + 1) * P)
                gt = pool.tile([P, N], in_dt, tag="g")
                xt = pool.tile([P, N], in_dt, tag="x")
                # The scan's internal state is fp32 regardless of operand
                # dtype; with a separate fp32 output tile the only bf16 loss
                # is input quantization. fp32 path scans in place (the DVE
                # write trails the read by the pipeline depth).
                if USE_BF16:
                    ot = pool.tile([P, N], mybir.dt.float32, tag="o", name="ot")
                else:
                    ot = xt
                # gates ride the sync HWDGE ring, inputs the scalar ring —
                # each chunk's pair transfers concurrently, and per-ring
                # FIFO drain makes completions arrive in scan order.
                nc.sync.dma_start(out=gt[:, :], in_=g[rows, :])
                nc.scalar.dma_start(out=xt[:, :], in_=x[rows, :])
                c0 = 0
                for seg in segs:
                    cols = slice(c0, c0 + seg)
                    init = 0.0 if c0 == 0 else ot[:, c0 - 1 : c0]
                    nc.vector.tensor_tensor_scan(
                        ot[:, cols],
                        gt[:, cols],
                        xt[:, cols],
                        init,
                        mybir.AluOpType.mult,
                        mybir.AluOpType.add,
                    )
                    # Stores take the (otherwise idle) GpSimd SWDGE path so
                    # their waits never stall either load ring.
                    nc.gpsimd.dma_start(out=o[rows, cols], in_=ot[:, cols])
                    c0 += seg
    nc.compile()
    return nc


def kernel(gates: np.ndarray, inputs: np.ndarray) -> np.ndarray:
    gates = np.asarray(gates, dtype=np.float32)
    inputs = np.asarray(inputs, dtype=np.float32)

    # Host-side shard: (B, N, D) -> lane-major (B*D, N); row b*D + d is the
    # contiguous time series of lane (b, d).
    gt = np.ascontiguousarray(gates.transpose(0, 2, 1)).reshape(LANES, N)
    xt = np.ascontiguousarray(inputs.transpose(0, 2, 1)).reshape(LANES, N)
    if USE_BF16:
        import ml_dtypes

        gt = gt.astype(ml_dtypes.bfloat16)
        xt = xt.astype(ml_dtypes.bfloat16)

    in_maps = []
    for c in range(N_CORES):
        rows = slice(c * LANES_PER_CORE, (c + 1) * LANES_PER_CORE)
        in_maps.append({"gates": gt[rows], "inputs": xt[rows]})

    nc = _build()
    res = run_bass_kernel_spmd(
        nc, in_maps, core_ids=list(range(N_CORES)), trace=TRACE
    )
    _result_info["exec_time_ns"] = res.exec_time_ns
    _result_info["mean_exec_time_ns"] = res.mean_exec_time_ns
    _result_info["profile_json"] = res.profile_json
    _result_info["trace"] = (
        res.instructions_and_trace[1] if res.instructions_and_trace else None
    )

    out_t = np.concatenate([r["out"] for r in res.results], axis=0)  # (LANES, N)
    return np.ascontiguousarray(out_t.reshape(B, D, N).transpose(0, 2, 1))


# revision 25
# speedup vs baseline: 1.1174x; 1.0417x over previous
"""AssocScan Trainium2 kernel: out[:, t] = gates[:, t] * out[:, t-1] + inputs[:, t].

Strategy: the recurrence is independent per (b, d) lane (B*D = 4096 lanes,
N = 4096 steps). The DVE `tensor_tensor_scan` instruction computes exactly
this recurrence along the free dimension, one lane per partition.

Sharding: lanes are split evenly across the 8 NeuronCores (512 lanes each).
During host-side sharding the (B, N, D) inputs are transposed to lane-major
(B*D, N) so every device DMA is fully contiguous (time series per lane
contiguous in DRAM); each core loads its shard with two 8 MiB DMAs, scans
4 tiles of [128 lanes, 4096 steps] in place on the VectorEngine, and
streams the results back.
"""

import sys

import numpy as np

for _p in ("/opt/trn_rl_repo", "/opt/pypackages"):
    if _p not in sys.path:
        sys.path.append(_p)

import concourse.bacc as bacc
import concourse.mybir as mybir
from concourse.bass_utils import run_bass_kernel_spmd
from concourse.tile import TileContext

B, N, D = 4, 4096, 1024
N_CORES = 8
LANES = B * D                        # 4096 independent (b, d) lanes
LANES_PER_CORE = LANES // N_CORES    # 512
P = 128                              # SBUF partitions
TILES_PER_CORE = LANES_PER_CORE // P # 4

TRACE = False       # test harness sets True to capture a neuron-profile trace
USE_BF16 = False    # bf16 input storage (halves load bytes); fp32 scan state
BF16_OUT = False    # dev knob: bf16 scan output tile + stores
_result_info = {}   # exec_time_ns / trace path from the last run

# Scan/store segment sizes along N per chunk. Loads are always whole-chunk
# (small DMAs drop well below line rate). The last chunk's final segments
# are smaller to shorten the exposed store tail after the last scan.
_PLAN = [
    [2048, 2048],
    [2048, 2048],
    [2048, 2048],
    [2048, 1024, 1024],
]


def _build() -> bacc.Bacc:
    in_dt = mybir.dt.bfloat16 if USE_BF16 else mybir.dt.float32
    nc = bacc.Bacc()
    g = nc.dram_tensor("gates", [LANES_PER_CORE, N], in_dt, kind="ExternalInput")
    x = nc.dram_tensor("inputs", [LANES_PER_CORE, N], in_dt, kind="ExternalInput")
    out_dt = mybir.dt.bfloat16 if BF16_OUT else mybir.dt.float32
    o = nc.dram_tensor("out", [LANES_PER_CORE, N], out_dt, kind="ExternalOutput")
    with TileContext(nc) as tc:
        with tc.tile_pool(name="pool", bufs=3) as pool:
            for i, segs in enumerate(_PLAN):
                rows = slice(i * P, (i + 1) * P)
                gt = pool.tile([P, N], in_dt, tag="g")
                xt = pool.tile([P, N], in_dt, tag="x")
                # The scan's internal state is fp32 regardless of operand
                # dtype; with a separate fp32 output tile the only bf16 loss
                # is input quantization. fp32 path scans in place (the DVE
                # write trails the read by the pipeline depth).
                if USE_BF16:
                    ot = pool.tile([P, N], out_dt, tag="o", name="ot")
                else:
                    ot = xt
                # Interleaved loads on the sync-engine HWDGE ring: FIFO
                # drain makes completions arrive in exactly scan order, so
                # each scan segment waits only for its own bytes.
                nc.sync.dma_start(out=gt[:, :], in_=g[rows, :])
                nc.sync.dma_start(out=xt[:, :], in_=x[rows, :])
                c0 = 0
                for seg in segs:
                    cols = slice(c0, c0 + seg)
                    init = 0.0 if c0 == 0 else ot[:, c0 - 1 : c0]
                    nc.vector.tensor_tensor_scan(
                        ot[:, cols],
                        gt[:, cols],
                        xt[:, cols],
                        init,
                        mybir.AluOpType.mult,
                        mybir.AluOpType.add,
                    )
                    # Stores ride the scalar-engine HWDGE ring so their
                    # waits never stall load issue on the sync ring.
                    nc.scalar.dma_start(out=o[rows, cols], in_=ot[:, cols])
                    c0 += seg
    nc.compile()
    return nc


def kernel(gates: np.ndarray, inputs: np.ndarray) -> np.ndarray:
    gates = np.asarray(gates, dtype=np.float32)
    inputs = np.asarray(inputs, dtype=np.float32)

    # Host-side shard: (B, N, D) -> lane-major (B*D, N); row b*D + d is the
    # contiguous time series of lane (b, d).
    gt = np.ascontiguousarray(gates.transpose(0, 2, 1)).reshape(LANES, N)
    xt = np.ascontiguousarray(inputs.transpose(0, 2, 1)).reshape(LANES, N)
    if USE_BF16:
        import ml_dtypes

        gt = gt.astype(ml_dtypes.bfloat16)
        xt = xt.astype(ml_dtypes.bfloat16)

    in_maps = []
    for c in range(N_CORES):
        rows = slice(c * LANES_PER_CORE, (c + 1) * LANES_PER_CORE)
        in_maps.append({"gates": gt[rows], "inputs": xt[rows]})

    nc = _build()
    res = run_bass_kernel_spmd(
        nc, in_maps, core_ids=list(range(N_CORES)), trace=TRACE
    )
    _result_info["exec_time_ns"] = res.exec_time_ns
    _result_info["mean_exec_time_ns"] = res.mean_exec_time_ns
    _result_info["profile_json"] = res.profile_json
    _result_info["trace"] = (
        res.instructions_and_trace[1] if res.instructions_and_trace else None
    )

    out_t = np.concatenate([r["out"] for r in res.results], axis=0)  # (LANES, N)
    out_t = out_t.astype(np.float32, copy=False)
    return np.ascontiguousarray(out_t.reshape(B, D, N).transpose(0, 2, 1))
